# revision 4
# baseline (speedup 1.0000x reference)
"""Trainium2 Bass kernel for nn_Net_25254407701209 (dense_mlp).

Reference math (B=4096, N_OBS=64, H=256):
    z   = x w1^T + b1;  y1 = z^2
    z12 = y1 w2a^T + b2a; z22 = y1 w2b^T + b2b; y2 = z12*z22
    z13 = y2 w3a^T + b3a; z23 = x w3s^T + b3s;  y3 = z13*z23
    b   = y3 wout^T;  yy = scalar broadcast
The reference's full-Jacobian chain collapses to a forward-mode JVP with
tangent xdot:
    v1 = 2 z * (xdot w1^T)
    t1 = z12 * (v1 w2b^T);  t2 = z22 * (v1 w2a^T)      (v2 = t1 + t2)
    g  = w3a v2 = w3a t1 + w3a t2     <- add folded into PSUM accumulation
    t3 = z13 * (xdot w3s^T);  t4 = z23 * g             (v3 = t3 + t4)
    bdot = wout . v3 = wout.t3 + wout.t4               <- folded likewise

Sharding: pure data-parallel, batch 4096 -> 8 cores x 512 rows. On-chip
layout is feature-on-partition: every [512, 256] activation is one
[128, 1024] tile (free block m holds features 128m..128m+127 for all 512
batch rows). Matmuls: lhsT = W^T stationary, rhs = activation, N=512,
float32r (1 cycle/row). Biases are folded into the PSUM->SBUF evictions
(ACT func(in+bias) / DVE tensor_scalar add). yy is a pure input-independent
broadcast and is assembled host-side.
"""
import sys

if "/opt/trn_rl_repo" not in sys.path:
    sys.path.insert(0, "/opt/trn_rl_repo")

import numpy as np

N_CORES = 8
B, NOBS, H = 4096, 64, 256
BL = B // N_CORES  # 512 batch rows per core
P = 128
N = BL

TRACE = False
TRACE_KW = {}
LAST = None
_CACHE = {}


def _build():
    import concourse.bacc as bacc
    import concourse.mybir as mybir
    import concourse.tile as tile

    F32 = mybir.dt.float32
    F32R = mybir.dt.float32r
    AF = mybir.ActivationFunctionType
    MUL = mybir.AluOpType.mult
    ADD = mybir.AluOpType.add

    nc = bacc.Bacc("TRN2", target_bir_lowering=False, debug=False, num_devices=N_CORES)

    d_xt = nc.dram_tensor("xt", [NOBS, N], F32R, kind="ExternalInput").ap()
    d_xdt = nc.dram_tensor("xdt", [NOBS, N], F32R, kind="ExternalInput").ap()
    d_w1t = nc.dram_tensor("w1t", [NOBS, H], F32R, kind="ExternalInput").ap()
    d_w3st = nc.dram_tensor("w3st", [NOBS, H], F32R, kind="ExternalInput").ap()
    d_w2at = nc.dram_tensor("w2at", [P, 2 * H], F32R, kind="ExternalInput").ap()
    d_w2bt = nc.dram_tensor("w2bt", [P, 2 * H], F32R, kind="ExternalInput").ap()
    d_w3at = nc.dram_tensor("w3at", [P, 2 * H], F32R, kind="ExternalInput").ap()
    d_woutt = nc.dram_tensor("woutt", [P, 2], F32R, kind="ExternalInput").ap()
    d_bc = nc.dram_tensor("bc", [P, 10], F32, kind="ExternalInput").ap()

    d_y = nc.dram_tensor("ydram", [P, 2 * N], F32R, kind="ExternalOutput").ap()
    d_b = nc.dram_tensor("bdram", [1, N], F32, kind="ExternalOutput").ap()
    d_bd = nc.dram_tensor("bddram", [1, N], F32, kind="ExternalOutput").ap()

    # bias column index in d_bc for (tensor j, half m): 2*j + m
    # j: 0=b1, 1=b2a, 2=b2b, 3=b3a, 4=b3s
    MSL = [slice(0, P), slice(P, 2 * P)]       # lhsT column slices per half
    FSL = [slice(0, N), slice(N, 2 * N)]       # free-block slices per half

    with tile.TileContext(nc) as tc:
        with (
            tc.tile_pool(name="io", bufs=1) as io,
            tc.tile_pool(name="act", bufs=1) as act,
            tc.tile_pool(name="psum", bufs=4, space="PSUM") as pp,
        ):
            # ---- input DMAs ----
            t_xt = io.tile([NOBS, N], F32R, name="t_xt")
            t_xdt = io.tile([NOBS, N], F32R, name="t_xdt")
            t_w1t = io.tile([NOBS, H], F32R, name="t_w1t")
            t_w3st = io.tile([NOBS, H], F32R, name="t_w3st")
            t_bc = io.tile([P, 10], F32, name="t_bc")
            nc.sync.dma_start(out=t_xt[:], in_=d_xt[:])
            nc.sync.dma_start(out=t_xdt[:], in_=d_xdt[:])
            nc.sync.dma_start(out=t_w1t[:], in_=d_w1t[:])
            nc.sync.dma_start(out=t_w3st[:], in_=d_w3st[:])
            nc.sync.dma_start(out=t_bc[:], in_=d_bc[:])
            t_w2at = io.tile([P, 2 * H], F32R, name="t_w2at")
            t_w2bt = io.tile([P, 2 * H], F32R, name="t_w2bt")
            t_w3at = io.tile([P, 2 * H], F32R, name="t_w3at")
            t_woutt = io.tile([P, 2], F32R, name="t_woutt")
            nc.sync.dma_start(out=t_w2at[:], in_=d_w2at[:])
            nc.sync.dma_start(out=t_w2bt[:], in_=d_w2bt[:])
            nc.sync.dma_start(out=t_w3at[:], in_=d_w3at[:])
            nc.sync.dma_start(out=t_woutt[:], in_=d_woutt[:])

            def bcol(j, m):
                return t_bc[:, 2 * j + m:2 * j + m + 1]

            # ---- phase 1: K=64 matmuls off x / xdot ----
            p_z = pp.tile([P, 2 * N], F32, name="p_z", tag="ps")
            p_u1 = pp.tile([P, 2 * N], F32, name="p_u1", tag="ps")
            p_z23 = pp.tile([P, 2 * N], F32, name="p_z23", tag="ps")
            p_u3s = pp.tile([P, 2 * N], F32, name="p_u3s", tag="ps")
            for m in range(2):
                nc.tensor.matmul(p_z[:, FSL[m]], t_w1t[:, MSL[m]], t_xt[:],
                                 start=True, stop=True)
                nc.tensor.matmul(p_u1[:, FSL[m]], t_w1t[:, MSL[m]], t_xdt[:],
                                 start=True, stop=True)
                nc.tensor.matmul(p_z23[:, FSL[m]], t_w3st[:, MSL[m]], t_xt[:],
                                 start=True, stop=True)
                nc.tensor.matmul(p_u3s[:, FSL[m]], t_w3st[:, MSL[m]], t_xdt[:],
                                 start=True, stop=True)

            t_y1 = act.tile([P, 2 * N], F32R, name="t_y1")
            t_ze = act.tile([P, 2 * N], F32, name="t_ze")
            t_z23e = act.tile([P, 2 * N], F32, name="t_z23e")
            for m in range(2):
                nc.scalar.activation(t_y1[:, FSL[m]], p_z[:, FSL[m]], AF.Square,
                                     bias=bcol(0, m), scale=1.0)
                nc.scalar.activation(t_ze[:, FSL[m]], p_z[:, FSL[m]], AF.Identity,
                                     bias=bcol(0, m), scale=1.0)
                nc.vector.tensor_scalar(t_z23e[:, FSL[m]], p_z23[:, FSL[m]],
                                        bcol(4, m), None, op0=ADD)
            t_v1 = act.tile([P, 2 * N], F32R, name="t_v1")
            nc.vector.scalar_tensor_tensor(t_v1[:], t_ze[:], 2.0, p_u1[:],
                                           op0=MUL, op1=MUL)

            # ---- phase 2: layer 2 (K=256 via two 128-halves) ----
            p_z12 = pp.tile([P, 2 * N], F32, name="p_z12", tag="ps")
            p_z22 = pp.tile([P, 2 * N], F32, name="p_z22", tag="ps")
            p_a = pp.tile([P, 2 * N], F32, name="p_a", tag="ps")
            p_b = pp.tile([P, 2 * N], F32, name="p_b", tag="ps")
            for m in range(2):
                nc.tensor.matmul(p_z12[:, FSL[m]], t_w2at[:, MSL[m]], t_y1[:, FSL[0]],
                                 start=True, stop=False)
                nc.tensor.matmul(p_z12[:, FSL[m]], t_w2at[:, H + MSL[m].start:H + MSL[m].stop],
                                 t_y1[:, FSL[1]], start=False, stop=True)
                nc.tensor.matmul(p_z22[:, FSL[m]], t_w2bt[:, MSL[m]], t_y1[:, FSL[0]],
                                 start=True, stop=False)
                nc.tensor.matmul(p_z22[:, FSL[m]], t_w2bt[:, H + MSL[m].start:H + MSL[m].stop],
                                 t_y1[:, FSL[1]], start=False, stop=True)
                nc.tensor.matmul(p_a[:, FSL[m]], t_w2bt[:, MSL[m]], t_v1[:, FSL[0]],
                                 start=True, stop=False)
                nc.tensor.matmul(p_a[:, FSL[m]], t_w2bt[:, H + MSL[m].start:H + MSL[m].stop],
                                 t_v1[:, FSL[1]], start=False, stop=True)
                nc.tensor.matmul(p_b[:, FSL[m]], t_w2at[:, MSL[m]], t_v1[:, FSL[0]],
                                 start=True, stop=False)
                nc.tensor.matmul(p_b[:, FSL[m]], t_w2at[:, H + MSL[m].start:H + MSL[m].stop],
                                 t_v1[:, FSL[1]], start=False, stop=True)

            t_z12e = act.tile([P, 2 * N], F32, name="t_z12e")
            t_z22e = act.tile([P, 2 * N], F32, name="t_z22e")
            for m in range(2):
                nc.scalar.activation(t_z12e[:, FSL[m]], p_z12[:, FSL[m]], AF.Identity,
                                     bias=bcol(1, m), scale=1.0)
                nc.vector.tensor_scalar(t_z22e[:, FSL[m]], p_z22[:, FSL[m]],
                                        bcol(2, m), None, op0=ADD)
            t_y2 = act.tile([P, 2 * N], F32R, name="t_y2")
            t_t1 = act.tile([P, 2 * N], F32R, name="t_t1")
            t_t2 = act.tile([P, 2 * N], F32R, name="t_t2")
            nc.gpsimd.tensor_mul(t_y2[:], t_z12e[:], t_z22e[:])
            nc.vector.tensor_mul(t_t1[:], t_z12e[:], p_a[:])
            nc.vector.tensor_mul(t_t2[:], t_z22e[:], p_b[:])

            # ---- phase 3: layer 3 ----
            p_z13 = pp.tile([P, 2 * N], F32, name="p_z13", tag="ps")
            p_g = pp.tile([P, 2 * N], F32, name="p_g", tag="ps")
            for m in range(2):
                nc.tensor.matmul(p_z13[:, FSL[m]], t_w3at[:, MSL[m]], t_y2[:, FSL[0]],
                                 start=True, stop=False)
                nc.tensor.matmul(p_z13[:, FSL[m]], t_w3at[:, H + MSL[m].start:H + MSL[m].stop],
                                 t_y2[:, FSL[1]], start=False, stop=True)
                nc.tensor.matmul(p_g[:, FSL[m]], t_w3at[:, MSL[m]], t_t1[:, FSL[0]],
                                 start=True, stop=False)
                nc.tensor.matmul(p_g[:, FSL[m]], t_w3at[:, H + MSL[m].start:H + MSL[m].stop],
                                 t_t1[:, FSL[1]], start=False, stop=False)
                nc.tensor.matmul(p_g[:, FSL[m]], t_w3at[:, MSL[m]], t_t2[:, FSL[0]],
                                 start=False, stop=False)
                nc.tensor.matmul(p_g[:, FSL[m]], t_w3at[:, H + MSL[m].start:H + MSL[m].stop],
                                 t_t2[:, FSL[1]], start=False, stop=True)

            t_z13e = act.tile([P, 2 * N], F32, name="t_z13e")
            for m in range(2):
                nc.scalar.activation(t_z13e[:, FSL[m]], p_z13[:, FSL[m]], AF.Identity,
                                     bias=bcol(3, m), scale=1.0)
            t_y3 = act.tile([P, 2 * N], F32R, name="t_y3")
            t_t3 = act.tile([P, 2 * N], F32R, name="t_t3")
            t_t4 = act.tile([P, 2 * N], F32R, name="t_t4")
            nc.gpsimd.tensor_mul(t_y3[:], t_z13e[:], t_z23e[:])
            nc.vector.tensor_mul(t_t3[:], t_z13e[:], p_u3s[:])
            nc.vector.tensor_mul(t_t4[:], t_z23e[:], p_g[:])
            nc.sync.dma_start(out=d_y[:], in_=t_y3[:])

            # ---- phase 4: wout contractions (M=1) ----
            p_bout = pp.tile([1, N], F32, name="p_bout", tag="ps")
            p_bd = pp.tile([1, N], F32, name="p_bd", tag="ps")
            nc.tensor.matmul(p_bout[0:1, :], t_woutt[:, 0:1], t_y3[:, FSL[0]],
                             start=True, stop=False)
            nc.tensor.matmul(p_bout[0:1, :], t_woutt[:, 1:2], t_y3[:, FSL[1]],
                             start=False, stop=True)
            nc.tensor.matmul(p_bd[0:1, :], t_woutt[:, 0:1], t_t3[:, FSL[0]],
                             start=True, stop=False)
            nc.tensor.matmul(p_bd[0:1, :], t_woutt[:, 1:2], t_t3[:, FSL[1]],
                             start=False, stop=False)
            nc.tensor.matmul(p_bd[0:1, :], t_woutt[:, 0:1], t_t4[:, FSL[0]],
                             start=False, stop=False)
            nc.tensor.matmul(p_bd[0:1, :], t_woutt[:, 1:2], t_t4[:, FSL[1]],
                             start=False, stop=True)

            t_brow = act.tile([1, N], F32, name="t_brow")
            t_bdrow = act.tile([1, N], F32, name="t_bdrow")
            nc.scalar.copy(t_brow[:], p_bout[0:1, :])
            nc.scalar.copy(t_bdrow[:], p_bd[0:1, :])
            nc.sync.dma_start(out=d_b[:], in_=t_brow[:])
            nc.sync.dma_start(out=d_bd[:], in_=t_bdrow[:])

    nc.compile()
    return nc


def kernel(x, xdot, w1, b1, w2a, b2a, w2b, b2b, w3a, b3a, w3s, b3s, wout, scalar):
    from concourse.bass_utils import run_bass_kernel_spmd

    global LAST
    if "nc" not in _CACHE:
        _CACHE["nc"] = _build()
    nc = _CACHE["nc"]

    f = np.float32
    x = np.asarray(x, f)
    xdot = np.asarray(xdot, f)
    sval = np.asarray(scalar, f).reshape(-1)[0]

    xt_full = np.ascontiguousarray(x.T)
    xdt_full = np.ascontiguousarray(xdot.T)

    def ksplit(w):  # [H, H] -> [128, 512] (k0 | k1)
        wt = np.asarray(w, f).T
        return np.ascontiguousarray(np.concatenate([wt[:P], wt[P:]], axis=1))

    w1t = np.ascontiguousarray(np.asarray(w1, f).T)
    w3st = np.ascontiguousarray(np.asarray(w3s, f).T)
    w2at, w2bt, w3at = ksplit(w2a), ksplit(w2b), ksplit(w3a)
    wo = np.asarray(wout, f)[0]
    woutt = np.ascontiguousarray(np.stack([wo[:P], wo[P:]], axis=1))
    bs = [np.asarray(a, f) for a in (b1, b2a, b2b, b3a, b3s)]
    bc = np.ascontiguousarray(
        np.stack([b[m * P:(m + 1) * P] for b in bs for m in range(2)], axis=1))

    in_maps = []
    for c in range(N_CORES):
        sl = slice(c * BL, (c + 1) * BL)
        in_maps.append({
            "xt": np.ascontiguousarray(xt_full[:, sl]),
            "xdt": np.ascontiguousarray(xdt_full[:, sl]),
            "w1t": w1t, "w3st": w3st,
            "w2at": w2at, "w2bt": w2bt, "w3at": w3at,
            "woutt": woutt, "bc": bc,
        })

    res = run_bass_kernel_spmd(
        nc, in_maps, core_ids=list(range(N_CORES)),
        trace=TRACE, **TRACE_KW)
    LAST = res

    yb = np.empty((B, 1), f)
    ybdot = np.empty((B,), f)
    yfull = np.empty((B, H), f)
    for c in range(N_CORES):
        sl = slice(c * BL, (c + 1) * BL)
        r = res.results[c]
        yb[sl, 0] = r["bdram"][0]
        ybdot[sl] = r["bddram"][0]
        yfull[sl, :P] = r["ydram"][:, :BL].T
        yfull[sl, P:] = r["ydram"][:, BL:].T
    yyfull = np.broadcast_to(np.float32(sval), (B, NOBS)).copy()
    yyfull += x * 0  # matches reference's x*0 + scalar exactly
    return yb, ybdot, yfull, yyfull
